# revision 6
# baseline (speedup 1.0000x reference)
"""Trainium2 Bass kernel for nn_EntDecoder v4: fp16 + slack-batched taps.

One group per core (groups are independent networks), 79 serial
anti-diagonal columns per group; critical path is ~24 cross-engine hops
per column: tap-1 -> relu0 -> [d0 -> pointwise]*10 -> mu -> head.
Measured ~560 ns per PE<->DVE hop pair on HW, which sets the floor.

1. All matmul operands (weights + activation planes + xb) are fp16:

  - fp32 InstMatmult is a self-loading 2-pass matmul (no LDWEIGHTS
    pull-ahead, 4 cycles/row).  fp16 lowers to LDWEIGHTS + MATMUL; the
    PE's 64-deep reorder window pulls the next layer's LDWEIGHTS ahead
    during the relu hop, and FWL (NumWeights==128) loads fast.
  - PSUM accumulation stays fp32; biases / thresholds / head compare
    stay fp32, so the only precision loss is fp16 rounding of weights
    and activations (11-bit mantissa; bf16 was validated off-line to
    flip decoded symbols, fp16 reproduces the fp32 decode exactly).

2. Off-chain conv taps are slack-batched into per-layer PSUM planes
   (packed 6-per-bank, reset per rep by a K=1 start=True zero matmul):
   tap d<0 for column c' is computable |d| columns early, so taps
   -2/-3/-4 issue every |d| columns as one N=|d| matmul.  Tap -1 (zero
   slack) issues inline in each layer's pointwise round-trip gap; the
   batches are deferred to the column boundary where the PE sits blocked
   for ~4 hops.  All pointwise ops run on the DVE (ACT's access
   latencies lose to DVE's at [128,1] sizes).

Decoded symbols are exact small ints (0..7) in fp16, so the xb feedback
path loses nothing.
"""

import numpy as np

NG, BN, H, W, K, NB = 8, 8, 32, 48, 5, 5
CBIAS = 3.5
NV = H + W - 1          # 79 diagonals
OFF = 4                 # left zero-pad columns in each plane
PW = OFF + NV           # plane width (83)
NL = 2 * NB             # hidden conv layers (10)

MM_DTYPE = "fp16"       # "fp16" | "bf16" | "f32"


# ---------------------------------------------------------------- host-side --
def _kmask(strict):
    d = np.arange(K) - K // 2
    s = d[:, None] + d[None, :]
    return ((s < 0) if strict else (s <= 0)).astype(np.float32)


_M0, _M1 = _kmask(True), _kmask(False)


def _taps(mask, d):
    out = []
    for di in range(-2, 3):
        dj = d - di
        if -2 <= dj <= 2 and mask[di + 2, dj + 2]:
            out.append((di, dj))
    return out


def _build_B0(w0g):
    """w0g [4,1,5,5] -> [4, 128, 32] B_d for d=-4..-1; out p=i*4+co, in p=i'."""
    B = np.zeros((4, 128, 32), np.float32)
    ii = np.arange(H)
    for k, d in enumerate(range(-4, 0)):
        for (di, dj) in _taps(_M0, d):
            iv = ii[(ii + di >= 0) & (ii + di < H)]
            for co in range(4):
                B[k, iv * 4 + co, iv + di] += w0g[co, 0, di + 2, dj + 2]
    return B


def _build_BH(wg):
    """wg [4,4,5,5] -> [5, 128, 128] B_d for d=-4..0 (i-major both sides)."""
    B = np.zeros((5, 128, 128), np.float32)
    ii = np.arange(H)
    for k, d in enumerate(range(-4, 1)):
        for (di, dj) in _taps(_M1, d):
            iv = ii[(ii + di >= 0) & (ii + di < H)]
            for co in range(4):
                for ci in range(4):
                    B[k, iv * 4 + co, (iv + di) * 4 + ci] += wg[co, ci, di + 2, dj + 2]
    return B


def _build_BL(wlg):
    """wlg [3,4,5,5] -> [5, 32, 128]: mu (param 1) only; out p=i, in p=i'*4+ci."""
    B = np.zeros((5, 32, 128), np.float32)
    ii = np.arange(H)
    for k, d in enumerate(range(-4, 1)):
        for (di, dj) in _taps(_M1, d):
            iv = ii[(ii + di >= 0) & (ii + di < H)]
            for ci in range(4):
                B[k, iv, (iv + di) * 4 + ci] += wlg[1, ci, di + 2, dj + 2]
    return B


def _split_waits(nc, max_waits=1):
    """walrus in this container rejects >1 sync waits on CTRL-class
    instructions (Tile's exit drain) -- hoist extras onto same-engine NOPs."""
    import concourse.mybir as mybir

    ctr = 0
    for fn in nc.m.functions:
        for bb in fn.blocks:
            out = []
            for ins in bb.instructions:
                sync = getattr(ins, "sync_info", None)
                if sync is not None and len(sync.on_wait) > max_waits:
                    waits = list(sync.on_wait)
                    extra, keep = waits[:-max_waits], waits[-max_waits:]
                    for s in range(0, len(extra), max_waits):
                        ctr += 1
                        nop = mybir.InstNoOp(
                            name=f"WSPLIT-{ctr}", text_hint="wait_split",
                            bass_nofuse=True)
                        nop.engine = ins.engine
                        nop.sync_info = mybir.SyncInfo(
                            on_wait=extra[s:s + max_waits], on_update=[])
                        out.append(nop)
                    ins.sync_info = mybir.SyncInfo(
                        on_wait=keep, on_update=list(sync.on_update))
                out.append(ins)
            bb.instructions = out
    return ctr


_ENG_SEM_PREFIX = {
    "EngineType.PE": "PE_",
    "EngineType.DVE": "DVE_",
    "EngineType.Activation": "ACT_",
    "EngineType.Pool": "POOL_",
}


def _drop_self_waits(nc):
    """Remove same-engine semaphore waits from dataflow engine instructions.

    Each engine executes its queue strictly in order, so a wait on the
    engine's own completion semaphore for an earlier instruction of the same
    queue is satisfied by construction.  Tile still emits them (its shadow
    model is conservative); with walrus limited to 1 wait/instruction each
    extra wait becomes a serialized NOP on the chain -- delete them instead.
    """
    import concourse.mybir as mybir

    ctr = 0
    for fn in nc.m.functions:
        for bb in fn.blocks:
            for ins in bb.instructions:
                tn = type(ins).__name__
                if tn in ("InstNoOp", "InstDrain", "InstSemDrain", "InstDMACopy"):
                    continue
                sync = getattr(ins, "sync_info", None)
                if sync is None or len(sync.on_wait) < 2:
                    continue
                pref = _ENG_SEM_PREFIX.get(str(ins.engine))
                if pref is None:
                    continue
                keep = [w for w in sync.on_wait
                        if not str(getattr(w, "ant_name", "")).startswith(pref)]
                if len(keep) != len(sync.on_wait):
                    ctr += len(sync.on_wait) - len(keep)
                    ins.sync_info = mybir.SyncInfo(
                        on_wait=keep, on_update=list(sync.on_update))
    return ctr


def _lohi(v):
    return max(0, v - (W - 1)), min(H - 1, v)


# ---------------------------------------------------------------- bass build --
def _build_nc(reps=1, relu_eng="dve", s_eng="dve", mm_dtype=None,
              force_loop=False, drop_offchain=False, same_weights=False,
              nblocks=NB):
    import concourse.bass as bass
    import concourse.mybir as mybir
    from concourse.tile import TileContext

    if mm_dtype is None:
        mm_dtype = MM_DTYPE
    fp32 = mybir.dt.float32
    adt = {"bf16": mybir.dt.bfloat16, "fp16": mybir.dt.float16,
           "f32": fp32}[mm_dtype]
    nc = bass.Bass()

    dW0 = nc.declare_dram_parameter("W0", [32, 4 * 128], adt, isOutput=False)
    dWH = nc.declare_dram_parameter("WH", [128, NL * 5 * 128], adt, isOutput=False)
    dWL = nc.declare_dram_parameter("WL", [128, 5 * 32], adt, isOutput=False)
    dBM0 = nc.declare_dram_parameter("BM0", [128, PW], fp32, isOutput=False)
    dBV = nc.declare_dram_parameter("BV", [128, NL * PW], fp32, isOutput=False)
    dNBV = nc.declare_dram_parameter("NBV", [128, NB * PW], fp32, isOutput=False)
    dTHP = nc.declare_dram_parameter("THP", [32, 8 * PW], fp32, isOutput=False)
    dXBIN = nc.declare_dram_parameter("XBIN", [32, PW], adt, isOutput=False)
    dOut = nc.declare_dram_parameter("XB", [32, PW], adt, isOutput=True)

    LAST = OFF + NV - 1     # last real column (82)

    with TileContext(nc) as tc:
        with (
            tc.tile_pool(name="const", bufs=1) as cpool,
            tc.tile_pool(name="planes", bufs=1) as ppool,
            tc.tile_pool(name="scratch", bufs=4) as spool,
            tc.tile_pool(name="psum", bufs=1, space="PSUM") as qpool,
        ):
            w0 = cpool.tile([32, 4 * 128], adt, tag="w0")
            wh = cpool.tile([128, NL * 5 * 128], adt, tag="wh")
            wl = cpool.tile([128, 5 * 32], adt, tag="wl")
            bm0 = cpool.tile([128, PW], fp32, tag="bm0")
            bv = cpool.tile([128, NL * PW], fp32, tag="bv")
            nbv = cpool.tile([128, NB * PW], fp32, tag="nbv")
            thp = cpool.tile([32, 8 * PW], fp32, tag="thp")

            nc.sync.dma_start(out=w0[:], in_=dW0[:])
            nc.sync.dma_start(out=wh[:], in_=dWH[:])
            nc.sync.dma_start(out=wl[:], in_=dWL[:])
            nc.sync.dma_start(out=bm0[:], in_=dBM0[:])
            nc.sync.dma_start(out=bv[:], in_=dBV[:])
            nc.sync.dma_start(out=nbv[:], in_=dNBV[:])
            nc.sync.dma_start(out=thp[:], in_=dTHP[:])

            # activation planes: xb [32,PW]; L0-out + per-block (h, x) [128,PW]
            xb = ppool.tile([32, PW], adt, tag="xb")
            planes = [ppool.tile([128, PW], adt, tag=f"pl{i}", name=f"pl{i}")
                      for i in range(11)]
            # xb init comes from DRAM (zeros normally; lets timing harnesses
            # chain executions back-to-back through a data dependency)
            nc.sync.dma_start(out=xb[:], in_=dXBIN[:])
            for p in planes:
                nc.vector.memset(p[:], 0.0)

            # PSUM planes, packed 6-per-bank: accumulation targets for the
            # slack-batched off-chain taps.  bankA: L0 + h1..h5; bankB:
            # h6..h10 + mu (mu on partitions 0:32 at column offset 5*PW).
            bankA = qpool.tile([128, 512], fp32, tag="bankA")
            bankB = qpool.tile([128, 512], fp32, tag="bankB")
            # zero-source rows for the per-rep PSUM reset matmuls (K=1)
            zrow = cpool.tile([1, 640], adt, tag="zrow")
            nc.vector.memset(zrow[:], 0.0)

            def psum_l(l):
                # l = 0: L0;  1..10: hidden;  11: mu
                if l <= 5:
                    return bankA[:, l * PW:(l + 1) * PW]
                if l <= 10:
                    return bankB[:, (l - 6) * PW:(l - 5) * PW]
                return bankB[0:32, 5 * PW:6 * PW]

            def mm(out_ap, lhsT_ap, rhs_ap, start, stop):
                nc.tensor.matmul(out_ap, lhsT_ap, rhs_ap, start=start,
                                 stop=stop, skip_group_check=True)

            def offchain_mm(out_ap, lhsT_ap, rhs_ap):
                # timing-experiment hook: optionally skip all off-chain taps
                if not drop_offchain:
                    mm(out_ap, lhsT_ap, rhs_ap, start=False, stop=False)

            add = mybir.AluOpType.add
            mult = mybir.AluOpType.mult
            vmax = mybir.AluOpType.max
            is_le = mybir.AluOpType.is_le
            Ident = mybir.ActivationFunctionType.Identity
            Relu = mybir.ActivationFunctionType.Relu

            def pointwise(out_ap, psum_ap, s1, s2, eng):
                # out = max(psum + s1, s2)
                if eng == "act" and isinstance(s2, float):
                    # plain relu (s2 == 0.0) can run on the scalar engine
                    nc.scalar.activation(
                        out=out_ap, in_=psum_ap, func=Relu, bias=s1, scale=1.0)
                else:
                    nc.vector.tensor_scalar(
                        out=out_ap, in0=psum_ap, scalar1=s1, scalar2=s2,
                        op0=add, op1=vmax)

            def emit_pad_rewrite():
                # serialize reps: rewrite the zero pad cols with a value
                # that depends on the previous rep's final columns.
                nc.vector.tensor_scalar(
                    out=xb[:, 0:OFF], in0=xb[:, PW - OFF:PW],
                    scalar1=0.0, scalar2=None, op0=mult)

            if force_loop:
                # hardware loop: same instruction footprint for any rep
                # count, so differential timing sees identical dispatch cost.
                # Body is unrolled x4 (4 serialized reps per iteration) to
                # halve the per-rep cost of the loop back-edge barrier.
                loop_ctx = tc.For_i(0, reps, 1)
                loop_ctx.__enter__()
                rep_range = [0, 1, 2, 3]
            else:
                loop_ctx = None
                rep_range = range(reps)

            for rep in rep_range:
                # per-rep PSUM reset: K=1 zero matmuls with start=True clear
                # the banks' has_written bits so all tap MMs accumulate with
                # start=False (tap groups interleave across columns).
                nc.tensor.matmul(bankA[:, :], zrow[:, 0:128], zrow[:, 128:640],
                                 start=True, stop=True, skip_group_check=True)
                nc.tensor.matmul(bankB[:, :], zrow[:, 0:128], zrow[:, 128:640],
                                 start=True, stop=True, skip_group_check=True)
                if rep > 0 or force_loop:
                    emit_pad_rewrite()
                for v in range(NV):
                    c = v + OFF
                    deferred = []

                    # ---- L0 (taps d=-4..-1 of xb; tap -1 gated on head(c-1)
                    # is the only on-chain MM; taps -2..-4 run in the relu0
                    # round-trip gap and never overlap column c).
                    pL0 = psum_l(0)
                    mm(pL0[:, c:c + 1], w0[:, 3 * 128:4 * 128],
                       xb[:, c - 1:c], start=False, stop=True)      # d=-1
                    pointwise(planes[0][:, c:c + 1], pL0[:, c:c + 1],
                              bm0[:, c:c + 1], 0.0, relu_eng)
                    if c + 1 <= LAST:                                # d=-2
                        offchain_mm(pL0[:, c + 1:c + 2],
                                    w0[:, 2 * 128:3 * 128], xb[:, c - 1:c])
                    if (c - 1) % 2 == 0:                             # d=-3
                        n = min(2, LAST - c)
                        if n > 0:
                            offchain_mm(pL0[:, c + 1:c + 1 + n],
                                         w0[:, 1 * 128:2 * 128],
                                         xb[:, c - 2:c - 2 + n])
                    if (c - 1) % 3 == 0:                             # d=-4
                        n = min(3, LAST - c)
                        if n > 0:
                            offchain_mm(pL0[:, c + 1:c + 1 + n],
                                         w0[:, 0 * 128:1 * 128],
                                         xb[:, c - 3:c - 3 + n])

                    # ---- hidden layers: d=0 on-chain (stop=True), then the
                    # pointwise, then off-chain taps for future columns.
                    xin = planes[0]
                    for b in range(nblocks):
                        for half in range(2):
                            l = 2 * b + half
                            base = l * 5 * 128
                            bcol = bv[:, l * PW + c:l * PW + c + 1]
                            pt = psum_l(1 + l)
                            if same_weights:
                                base = 0
                            mm(pt[:, c:c + 1],
                               wh[:, base + 4 * 128:base + 5 * 128],
                               xin[:, c:c + 1], start=False, stop=True)
                            if half == 0:
                                hpl = planes[1 + 2 * b]
                                pointwise(hpl[:, c:c + 1], pt[:, c:c + 1],
                                          bcol, 0.0, relu_eng)
                                xout = hpl
                            else:
                                xo = planes[2 + 2 * b]
                                xprev = planes[0] if b == 0 else planes[2 * b]
                                # s = bias + xprev, computed off-chain
                                # (bias is -1e30 on invalid rows -> s=-1e30)
                                s = spool.tile([128, 1], fp32, tag="s")
                                if s_eng == "act":
                                    nc.scalar.activation(
                                        out=s[:, :],
                                        in_=xprev[:, c:c + 1],
                                        func=Ident,
                                        bias=bcol, scale=1.0)
                                else:
                                    nc.vector.tensor_scalar(
                                        out=s[:, :],
                                        in0=xprev[:, c:c + 1],
                                        scalar1=bcol,
                                        scalar2=None, op0=add)
                                # xprev + relu(psum + bias)
                                #   = max(psum, -bias) + (xprev + bias)
                                # nbv valid rows: -bias; invalid: +1e30,
                                # so invalid rows give 1e30 + (-1e30) = 0.
                                nbcol = nbv[:, b * PW + c:b * PW + c + 1]
                                nc.vector.tensor_scalar(
                                    out=xo[:, c:c + 1], in0=pt[:, c:c + 1],
                                    scalar1=nbcol, scalar2=s[:, :],
                                    op0=vmax, op1=add)
                                xout = xo
                            # off-chain taps of this layer (targets > c):
                            if c + 1 <= LAST:                        # d=-1
                                offchain_mm(pt[:, c + 1:c + 2],
                                            wh[:, base + 3 * 128:base + 4 * 128],
                                            xin[:, c:c + 1])
                            for dd in (2, 3, 4):                     # d=-dd
                                if c % dd == 0:
                                    n = min(dd, LAST - c)
                                    if n > 0:
                                        deferred.append((
                                            pt[:, c + 1:c + 1 + n],
                                            wh[:, base + (4 - dd) * 128:
                                               base + (5 - dd) * 128],
                                            xin[:, c + 1 - dd:c + 1 - dd + n]))
                            xin = xout

                    # ---- mu layer ([32] out) + head
                    pm = psum_l(11)
                    mm(pm[:, c:c + 1], wl[:, 4 * 32:5 * 32],
                       xin[:, c:c + 1], start=False, stop=True)
                    # head: one DVE tensor_scalar with accum reduce:
                    # xb[:,c] = sum_k 1[thr_k <= mu]  (raw symbol 0..7)
                    scr = spool.tile([32, 8], fp32, tag="scr")
                    nc.vector.tensor_scalar(
                        out=scr[:, :], in0=thp[:, c * 8:(c + 1) * 8],
                        scalar1=pm[:, c:c + 1], scalar2=None,
                        op0=is_le, op1=add,
                        accum_out=xb[:, c:c + 1])
                    if c + 1 <= LAST:                                # d=-1
                        offchain_mm(pm[:, c + 1:c + 2], wl[:, 3 * 32:4 * 32],
                                    xin[:, c:c + 1])
                    for dd in (2, 3, 4):                             # d=-dd
                        if c % dd == 0:
                            n = min(dd, LAST - c)
                            if n > 0:
                                deferred.append((
                                    pm[:, c + 1:c + 1 + n],
                                    wl[:, (4 - dd) * 32:(5 - dd) * 32],
                                    xin[:, c + 1 - dd:c + 1 - dd + n]))
                    # batched taps execute in the column-boundary gap while
                    # the PE is blocked on head(c) -> L0 tap -1 -> relu0.
                    for (o, w, r) in deferred:
                        offchain_mm(o, w, r)

            if loop_ctx is not None:
                loop_ctx.__exit__(None, None, None)
            nc.sync.dma_start(out=dOut[:], in_=xb[:])

    _drop_self_waits(nc)
    _split_waits(nc)
    return nc


# ------------------------------------------------------------------- kernel --
def make_in_maps(mask, w0, wb, bb, w_last, b_last, mm_dtype=None):
    import ml_dtypes

    if mm_dtype is None:
        mm_dtype = MM_DTYPE
    wdt = {"bf16": ml_dtypes.bfloat16, "fp16": np.float16,
           "f32": np.float32}[mm_dtype]

    w0 = np.asarray(w0, np.float32)
    wb = np.asarray(wb, np.float32)
    bb = np.asarray(bb, np.float32)
    w_last = np.asarray(w_last, np.float32)
    b_last = np.asarray(b_last, np.float32)
    mask = np.asarray(mask, np.float32)

    # validity: cell (i, v) is in-image iff lo(v) <= i <= hi(v)
    valid = np.zeros((H, PW), np.float32)
    for v in range(NV):
        lo, hi = _lohi(v)
        valid[lo:hi + 1, v + OFF] = 1.0

    in_maps = []
    for g in range(NG):
        # mask folded in: decoded-symbol cells are valid & unmasked
        vm = valid.copy()
        for v in range(NV):
            lo, hi = _lohi(v)
            for i in range(lo, hi + 1):
                vm[i, v + OFF] = valid[i, v + OFF] * mask[g, 0, i, v - i]

        B0 = _build_B0(w0[g])                      # [4,128,32]
        B0q = B0.astype(wdt).astype(np.float32)    # what the PE will see
        W0p = np.ascontiguousarray(
            B0.transpose(2, 0, 1).reshape(32, 4 * 128)).astype(wdt)
        BH = np.stack([_build_BH(wb[l, g]) for l in range(NL)])  # [NL,5,128,128]
        WHp = np.ascontiguousarray(
            BH.transpose(3, 0, 1, 2).reshape(128, NL * 5 * 128)).astype(wdt)
        BL = _build_BL(w_last[g])                  # [5,32,128]
        WLp = np.ascontiguousarray(
            BL.transpose(2, 0, 1).reshape(128, 5 * 32)).astype(wdt)
        # bias planes [128, NL*PW]: bias on geometrically-valid cells,
        # -1e30 on invalid (so max(psum+bias, 0/x) pins those rows to 0)
        valid128 = np.repeat(valid, 4, axis=0)            # [128, PW]
        bvec = np.tile(bb[:, g, :], (1, 32)).reshape(NL, 128)
        BVp = np.where(valid128[None, :, :] > 0,
                       bvec[:, :, None], np.float32(-1e30))
        BVp = np.ascontiguousarray(
            BVp.transpose(1, 0, 2).reshape(128, NL * PW).astype(np.float32))
        # resid pointwise scalar1: -bias on valid rows, +1e30 on invalid
        # (indexed by block: layer l = 2b+1)
        nbvec = bvec[1::2]                                # [NB, 128]
        NBVp = np.where(valid128[None, :, :] > 0,
                        -nbvec[:, :, None], np.float32(1e30))
        NBVp = np.ascontiguousarray(
            NBVp.transpose(1, 0, 2).reshape(128, NB * PW).astype(np.float32))
        # per-column thresholds [32, 8*PW]: k+1 - (b_mu+4) on valid&unmasked,
        # +inf else (masked cells decode to symbol 0 = never written)
        thr = np.arange(1, 9, dtype=np.float32) - (b_last[g, 1] + 4.0)
        thr[7] = 1e30
        THPp = np.full((H, PW, 8), 1e30, np.float32)
        THPp[vm > 0] = thr
        THPp = np.ascontiguousarray(THPp.reshape(H, PW * 8))
        # layer-0 bias plane: xb stores raw counts (0..7); the -3.5 centering
        # becomes a correction -3.5 * sum_d B0q_d @ vm(:, c+d)  (B0q = the
        # quantized tap matrices actually used by the PE).
        corr0 = np.zeros((128, PW), np.float32)
        for c in range(OFF, PW):
            acc = np.zeros(128, np.float32)
            for k in range(4):
                acc += B0q[k] @ vm[:, c - 4 + k]
            corr0[:, c] = CBIAS * acc
        BM0p = np.ascontiguousarray(
            np.where(valid128 > 0, -corr0, np.float32(-1e30)).astype(np.float32))
        in_maps.append({"W0": W0p, "WH": WHp, "WL": WLp,
                        "BM0": BM0p, "BV": BVp, "NBV": NBVp, "THP": THPp,
                        "XBIN": np.zeros((H, PW), wdt)})
    return in_maps


def kernel(mask, w0, wb, bb, w_last, b_last):
    from concourse.bass_utils import run_bass_kernel_spmd

    mask = np.asarray(mask, np.float32)
    nc = _build_nc()
    in_maps = make_in_maps(mask, w0, wb, bb, w_last, b_last)
    res = run_bass_kernel_spmd(nc, in_maps, core_ids=list(range(NG)))

    out = np.zeros((NG, 1, H, W), np.float32)
    ii = np.arange(H)[:, None]
    jj = np.arange(W)[None, :]
    for g in range(NG):
        xbp = np.asarray(res.results[g]["XB"], np.float32)  # [32, PW]
        out[g, 0] = xbp[ii, OFF + ii + jj]
    return out * mask
